# revision 17
# baseline (speedup 1.0000x reference)
"""Trainium2 Bass kernel for fused AdaRMSNorm + QK-RMSNorm/RoPE attention.

Sharding: 8 cores = 2 batch groups x 4 head-groups (8 heads each).
Device math (per core, batch b, head-group g):
  xs = x[b].T * rsqrt(mean_d(x^2)+eps)            (adaRMS: scale folded into W, shift into bias c)
  qT/kT = (W*.T)^T @ xs + c   (hd-major)           v = xs.T @ Wv* + cv  (L-major)
  per-head: qn = q * rsqrt(mean_hd(q^2)+eps); rope via partition pair-shuffle,
            qk_w/sign/(1/sqrt hd) folded into host rope tables
  S^T = kr^T qr ; E^T = exp(S^T) (no max-sub: scores are O(5))
  o+ = [v;1]^T E^T  -> oT = o/denom ; partial outT = W_o_loc @ oT
Host: final 4-way partial sum per batch + transpose.
"""

import numpy as np

B, L, D, HD, DC = 2, 2048, 2048, 64, 2048
NH = D // HD
EPS = float(np.finfo(np.float32).eps)
NCORES = 8
JL = 512          # local j per device (8 heads x 64)
NHL = 8           # local heads


# ---------------------------------------------------------------- host prep
def _host_prep(x, condition, rope, w_ada, w_qkv, w_out, qk_w):
    hd_idx = np.arange(HD)
    sign = np.where(hd_idx % 2 == 0, -1.0, 1.0).astype(np.float32)
    partner = hd_idx ^ 1
    rope0, rope1 = rope[0].T, rope[1].T                      # [64, L]
    ropeA = (rope0 * qk_w[:, None]).astype(np.float32)
    ropeB = (rope1 * (sign * qk_w[partner])[:, None]).astype(np.float32)
    scl = np.float32(1.0 / np.sqrt(HD))
    # 2-head replicated [128, L] tiles
    ropeA2 = np.tile(ropeA, (2, 1))
    ropeB2 = np.tile(ropeB, (2, 1))
    ra_q = np.ascontiguousarray(ropeA2 * scl)
    rb_q = np.ascontiguousarray(ropeB2 * scl)
    ra_k = np.ascontiguousarray(ropeA2)
    rb_k = np.ascontiguousarray(ropeB2)

    smat = np.zeros((JL, NHL), np.float32)                   # j -> head one-hot
    for j in range(JL):
        smat[j, j // HD] = 1.0
    sbmat = np.ascontiguousarray(smat.T)                     # [8, 512]

    in_maps = []
    for dev in range(NCORES):
        b, g = dev // 4, dev % 4
        ss = (w_ada @ condition[b]).astype(np.float32)
        shift, s1 = ss[:D], (1.0 + ss[D:]).astype(np.float32)
        Wq = w_qkv[g * JL:(g + 1) * JL]
        Wk = w_qkv[D + g * JL:D + (g + 1) * JL]
        Wv = w_qkv[2 * D + g * JL:2 * D + (g + 1) * JL]
        m = {
            "xT": np.ascontiguousarray(x[b].T),
            "wq": np.ascontiguousarray((Wq * s1[None, :]).T),
            "wk": np.ascontiguousarray((Wk * s1[None, :]).T),
            "wv": np.ascontiguousarray((Wv * s1[None, :]).T),
            "cq": (Wq @ shift).astype(np.float32).reshape(1, JL),
            "ck": (Wk @ shift).astype(np.float32).reshape(1, JL),
            "cv": (Wv @ shift).astype(np.float32).reshape(1, JL),
            "ra_q": ra_q, "rb_q": rb_q, "ra_k": ra_k, "rb_k": rb_k,
            "smat": smat, "sbmat": sbmat,
            "woT": np.ascontiguousarray(w_out[:, g * JL:(g + 1) * JL].T),  # [512 j, 2048 dcol]
        }
        in_maps.append(m)
    return in_maps


# ---------------------------------------------------------------- bass build
def _build_nc():
    import concourse.bass as bass
    import concourse.mybir as mybir
    import concourse.tile as tile
    from concourse import bacc

    f32 = mybir.dt.float32
    f32r = mybir.dt.float32r
    PS = 512
    AF = mybir.ActivationFunctionType

    nc = bacc.Bacc("TRN2", target_bir_lowering=False, debug=False, num_devices=8)
    xT_d = nc.dram_tensor("xT", [D, L], f32, kind="ExternalInput")
    w_d = {nm: nc.dram_tensor(nm, [D, JL], f32, kind="ExternalInput")
           for nm in ("wq", "wk", "wv")}
    c_d = {nm: nc.dram_tensor(f"c{nm}", [1, JL], f32, kind="ExternalInput")
           for nm in ("q", "k", "v")}
    rope_d = {nm: nc.dram_tensor(nm, [128, L], f32, kind="ExternalInput")
              for nm in ("ra_q", "rb_q", "ra_k", "rb_k")}
    smat_d = nc.dram_tensor("smat", [JL, NHL], f32, kind="ExternalInput")
    sbmat_d = nc.dram_tensor("sbmat", [NHL, JL], f32, kind="ExternalInput")
    woT_d = nc.dram_tensor("woT", [JL, D], f32, kind="ExternalInput")
    out_d = nc.dram_tensor("out", [D, L], f32, kind="ExternalOutput")

    import os
    USE_F32R = os.environ.get("USE_F32R", "0") == "1"

    def r(ap):
        return ap.bitcast(f32r) if USE_F32R else ap

    import contextlib
    lp = nc.allow_low_precision(reason="fp32r matmul operands") if USE_F32R \
        else contextlib.nullcontext()
    with tile.TileContext(nc) as tc, lp:
        with (
            tc.tile_pool(name="consts", bufs=1) as consts,
            tc.tile_pool(name="dram", bufs=1, space="DRAM") as dpool,
        ):
            ones_raw = consts.tile([128, PS], f32)
            nc.vector.memset(ones_raw, 1.0)
            ones = consts.tile([128, PS], f32)
            nc.vector.tensor_copy(r(ones), ones_raw)
            eps1 = consts.tile([1, 1], f32)
            nc.vector.memset(eps1, EPS)
            eps8 = consts.tile([8, 1], f32)
            nc.vector.memset(eps8, EPS)
            s_t = consts.tile([128, 4, NHL], f32)
            for i in range(4):
                nc.sync.dma_start(out=r(s_t[:, i, :]), in_=r(smat_d[i * 128:(i + 1) * 128, :]))
            sb_t = consts.tile([NHL, JL], f32)
            nc.sync.dma_start(out=r(sb_t), in_=r(sbmat_d[:, :]))
            c_t = {}
            for nm in ("q", "k", "v"):
                c_t[nm] = consts.tile([1, JL], f32, tag=f"c{nm}", name=f"c{nm}")
                nc.sync.dma_start(out=r(c_t[nm]), in_=r(c_d[nm][:, :]))
            rr_row = consts.tile([1, L], f32)

            qT_sp = dpool.tile([JL, L], f32, tag="qTsp")
            kT_sp = dpool.tile([JL, L], f32, tag="kTsp")
            v_sp = dpool.tile([L, NHL, HD + 1], f32, tag="vsp")

            # ================= Phase A/B/C: xs resident ====================
            with tc.tile_pool(name="xs", bufs=1) as xsp:
                xs = []
                for i in range(16):
                    t = xsp.tile([128, L], f32, tag=f"xs{i}", name=f"xs{i}")
                    nc.sync.dma_start(out=r(t), in_=r(xT_d[i * 128:(i + 1) * 128, :]))
                    xs.append(t)
                # ---- A: sumsq over d via ones-matmul; B: xs *= rr ----
                with (
                    tc.tile_pool(name="tmpA", bufs=2) as tmpA,
                    tc.tile_pool(name="psA", bufs=1, space="PSUM") as psA,
                ):
                    ssq = psA.tile([1, L], f32, tag="ssq")
                    touch = tmpA.tile([1, 16], f32, tag="touch", name="touch", bufs=2)
                    for i in range(16):
                        nc.vector.tensor_copy(touch[0:1, i:i + 1], xs[i][0:1, 0:1])
                        for hh in range(2):
                            sq = tmpA.tile([128, 1024], f32, tag="sq")
                            sl = slice(hh * 1024, (hh + 1) * 1024)
                            nc.vector.tensor_mul(r(sq), xs[i][:, sl], xs[i][:, sl])
                            for n2 in range(2):
                                nc.tensor.matmul(
                                    ssq[:, hh * 1024 + n2 * PS:hh * 1024 + (n2 + 1) * PS],
                                    lhsT=r(ones[:, 0:1]), rhs=r(sq[:, n2 * PS:(n2 + 1) * PS]),
                                    start=(i == 0), stop=(i == 15))
                    nc.scalar.activation(r(rr_row), ssq, AF.Sqrt, bias=eps1, scale=1.0 / D)
                    nc.vector.reciprocal(r(rr_row), rr_row)
                    rrb_ps = psA.tile([128, L], f32, tag="rrb")
                    for n in range(4):
                        nc.tensor.matmul(rrb_ps[:, n * PS:(n + 1) * PS],
                                         lhsT=r(ones[0:1, 0:128]),
                                         rhs=r(rr_row[:, n * PS:(n + 1) * PS]),
                                         start=True, stop=True)
                    for i in range(16):
                        nc.vector.tensor_mul(r(xs[i]), xs[i], rrb_ps)

                # ---- C1: q/k projections (hd-major), spill to DRAM ----
                with (
                    tc.tile_pool(name="wqk", bufs=1) as wqk,
                    tc.tile_pool(name="bnc", bufs=2) as bnc,
                    tc.tile_pool(name="psC", bufs=1, space="PSUM") as psC,
                ):
                    for nm, spill in (("q", qT_sp), ("k", kT_sp)):
                        for lh in range(2):
                            ps = [psC.tile([128, 1024], f32, tag=f"pj{m}", name=f"pj{m}")
                                  for m in range(4)]
                            for k16 in range(16):
                                wt = wqk.tile([128, JL], f32, tag=f"w{k16 % 8}",
                                              name=f"w{k16 % 8}")
                                nc.sync.dma_start(
                                    out=r(wt), in_=r(w_d["w" + nm][k16 * 128:(k16 + 1) * 128, :]))
                                for m in range(4):
                                    for n2 in range(2):
                                        nc.tensor.matmul(
                                            ps[m][:, n2 * PS:(n2 + 1) * PS],
                                            lhsT=r(wt[:, m * 128:(m + 1) * 128]),
                                            rhs=r(xs[k16][:, lh * 1024 + n2 * PS:
                                                          lh * 1024 + (n2 + 1) * PS]),
                                            start=(k16 == 0), stop=False)
                            for m in range(4):
                                for n2 in range(2):
                                    nc.tensor.matmul(
                                        ps[m][:, n2 * PS:(n2 + 1) * PS],
                                        lhsT=r(c_t[nm][:, m * 128:(m + 1) * 128]),
                                        rhs=r(ones[0:1, 0:PS]), start=False, stop=True)
                                o = bnc.tile([128, 1024], f32, tag="bounce")
                                nc.vector.tensor_copy(o, ps[m])
                                nc.sync.dma_start(
                                    out=spill[m * 128:(m + 1) * 128, lh * 1024:(lh + 1) * 1024],
                                    in_=o)

                # ---- C2: v projection (L-major), spill with ones column ----
                with (
                    tc.tile_pool(name="wv16", bufs=1) as wv16,
                    tc.tile_pool(name="bncv", bufs=2) as bncv,
                    tc.tile_pool(name="psV", bufs=2, space="PSUM") as psV,
                ):
                    wt = []
                    for k16 in range(16):
                        t = wv16.tile([128, JL], f32, tag=f"wv{k16}", name=f"wv{k16}")
                        nc.sync.dma_start(out=r(t), in_=r(w_d["wv"][k16 * 128:(k16 + 1) * 128, :]))
                        wt.append(t)
                    for mL in range(16):
                        pv = psV.tile([128, JL], f32, tag="pv")
                        for k16 in range(16):
                            nc.tensor.matmul(pv, lhsT=r(xs[k16][:, mL * 128:(mL + 1) * 128]),
                                             rhs=r(wt[k16]), start=(k16 == 0), stop=False)
                        nc.tensor.matmul(pv, lhsT=r(ones[0:1, 0:128]), rhs=r(c_t["v"]),
                                         start=False, stop=True)
                        vb = bncv.tile([128, NHL, HD + 1], f32, tag="vb")
                        nc.vector.tensor_copy(vb[:, :, 0:HD],
                                              pv.rearrange("p (h d) -> p h d", h=NHL))
                        nc.vector.memset(vb[:, :, HD:HD + 1], 1.0)
                        nc.sync.dma_start(out=v_sp[mL * 128:(mL + 1) * 128], in_=vb)

            # ================= Phase D/E/F =================================
            with tc.tile_pool(name="oT", bufs=1) as otp:
                oT = [otp.tile([128, L], f32, tag=f"oT{m}", name=f"oT{m}") for m in range(4)]
                with tc.tile_pool(name="qr", bufs=1) as qrp:
                    qr_t, kr_t = [], []
                    # ---- D: per-head RMS + RoPE ----
                    with (
                        tc.tile_pool(name="ropes", bufs=1) as rp,
                        tc.tile_pool(name="dtmp", bufs=2) as dtmp,
                        tc.tile_pool(name="psD", bufs=1, space="PSUM") as psD,
                    ):
                        rope_t = {}
                        for nm in ("ra_q", "rb_q", "ra_k", "rb_k"):
                            rope_t[nm] = rp.tile([128, L], f32, tag=nm, name=nm)
                            nc.sync.dma_start(out=rope_t[nm], in_=rope_d[nm][:, :])
                        shuf = [i ^ 1 for i in range(32)]
                        for is_q, spill, dst in ((1, qT_sp, qr_t), (0, kT_sp, kr_t)):
                            ra = rope_t["ra_q" if is_q else "ra_k"]
                            rb = rope_t["rb_q" if is_q else "rb_k"]
                            for m in range(4):
                                qk = dtmp.tile([128, L], f32, tag="tA", name="tA", bufs=2)
                                nc.sync.dma_start(out=qk, in_=spill[m * 128:(m + 1) * 128, :])
                                sq = dtmp.tile([128, L], f32, tag="tB", name="tB", bufs=2)
                                nc.scalar.square(r(sq), qk)
                                ph = psD.tile([NHL, L], f32, tag="ph")
                                for n in range(4):
                                    nc.tensor.matmul(ph[:, n * PS:(n + 1) * PS],
                                                     lhsT=r(s_t[:, m, :]),
                                                     rhs=r(sq[:, n * PS:(n + 1) * PS]),
                                                     start=True, stop=True)
                                rrh = dtmp.tile([NHL, L], f32, tag="tC", name="tC", bufs=1)
                                nc.scalar.activation(r(rrh), ph, AF.Sqrt, bias=eps8, scale=1.0 / HD)
                                nc.vector.reciprocal(r(rrh), rrh)
                                pb = psD.tile([128, L], f32, tag="pb")
                                for n in range(4):
                                    nc.tensor.matmul(pb[:, n * PS:(n + 1) * PS],
                                                     lhsT=r(sb_t[:, m * 128:(m + 1) * 128]),
                                                     rhs=r(rrh[:, n * PS:(n + 1) * PS]),
                                                     start=True, stop=True)
                                qn = dtmp.tile([128, L], f32, tag="tD", name="tD", bufs=1)
                                nc.vector.tensor_mul(qn, qk, pb)
                                qs = dtmp.tile([128, L], f32, tag="tB", name="qs", bufs=2)
                                nc.vector.stream_shuffle(qs, qn, shuf)
                                t1 = dtmp.tile([128, L], f32, tag="tC", name="t1", bufs=1)
                                nc.vector.tensor_mul(t1, qn, ra)
                                t2 = dtmp.tile([128, L], f32, tag="tA", name="t2", bufs=2)
                                nc.gpsimd.tensor_mul(t2, qs, rb)
                                res = qrp.tile([128, L], f32,
                                               tag=f"{'qr' if is_q else 'kr'}{m}",
                                               name=f"{'qr' if is_q else 'kr'}{m}")
                                nc.vector.tensor_add(r(res), t1, t2)
                                dst.append(res)

                    # ---- E: attention ----
                    with (
                        tc.tile_pool(name="vload", bufs=1) as vlp,
                        tc.tile_pool(name="et", bufs=3) as etp,
                        tc.tile_pool(name="etmp", bufs=2) as etmp,
                        tc.tile_pool(name="psS", bufs=2, space="PSUM") as psS,
                        tc.tile_pool(name="psO", bufs=1, space="PSUM") as psO,
                    ):
                        for h in range(NHL):
                            qr_h = qr_t[h // 2][(h % 2) * 64:(h % 2) * 64 + 64, :]
                            kr_h = kr_t[h // 2][(h % 2) * 64:(h % 2) * 64 + 64, :]
                            vt = []
                            for l2c in range(16):
                                t = vlp.tile([128, HD + 1], f32, tag=f"v{l2c}",
                                             name=f"v{l2c}")
                                nc.sync.dma_start(out=r(t),
                                                  in_=r(v_sp[l2c * 128:(l2c + 1) * 128, h, :]))
                                vt.append(t)
                            for lhf in range(2):
                                po = psO.tile([HD + 1, 1024], f32, tag="po")
                                for l2c in range(16):
                                    pS = psS.tile([128, 1024], f32, tag="pS", name="pS")
                                    for n2 in range(2):
                                        nc.tensor.matmul(
                                            pS[:, n2 * PS:(n2 + 1) * PS],
                                            lhsT=r(kr_h[:, l2c * 128:(l2c + 1) * 128]),
                                            rhs=r(qr_h[:, lhf * 1024 + n2 * PS:
                                                       lhf * 1024 + (n2 + 1) * PS]),
                                            start=True, stop=True)
                                    et = etp.tile([128, 1024], f32, tag="et", name="et")
                                    nc.scalar.activation(r(et), pS, AF.Exp)
                                    for n2 in range(2):
                                        nc.tensor.matmul(
                                            po[:, n2 * PS:(n2 + 1) * PS],
                                            lhsT=r(vt[l2c]),
                                            rhs=r(et[:, n2 * PS:(n2 + 1) * PS]),
                                            start=(l2c == 0), stop=(l2c == 15))
                                rd = etmp.tile([1, 1024], f32, tag="rd", name="rd")
                                nc.vector.reciprocal(r(rd), po[HD:HD + 1, :])
                                prd = psS.tile([128, 1024], f32, tag="pS", name="prd")
                                for n2 in range(2):
                                    nc.tensor.matmul(prd[0:HD, n2 * PS:(n2 + 1) * PS],
                                                     lhsT=r(ones[0:1, 0:HD]),
                                                     rhs=r(rd[:, n2 * PS:(n2 + 1) * PS]),
                                                     start=True, stop=True)
                                ob = etmp.tile([HD, 1024], f32, tag="ob", name="ob")
                                nc.scalar.copy(ob, po[0:HD, :])
                                nc.vector.tensor_mul(
                                    r(oT[h // 2][(h % 2) * 64:(h % 2) * 64 + 64,
                                                 lhf * 1024:(lhf + 1) * 1024]),
                                    ob, prd[0:HD, :])

                # ---- F: partial output projection ----
                with (
                    tc.tile_pool(name="wo", bufs=1) as wop,
                    tc.tile_pool(name="fbnc", bufs=2) as fbnc,
                    tc.tile_pool(name="psF", bufs=2, space="PSUM") as psF,
                ):
                    wo = []
                    for kj in range(4):
                        t = wop.tile([128, D], f32, tag=f"wo{kj}", name=f"wo{kj}")
                        nc.sync.dma_start(out=r(t), in_=r(woT_d[kj * 128:(kj + 1) * 128, :]))
                        wo.append(t)
                    for m16 in range(16):
                        pf = psF.tile([128, L], f32, tag="pf")
                        for kj in range(4):
                            for n in range(4):
                                nc.tensor.matmul(
                                    pf[:, n * PS:(n + 1) * PS],
                                    lhsT=r(wo[kj][:, m16 * 128:(m16 + 1) * 128]),
                                    rhs=r(oT[kj][:, n * PS:(n + 1) * PS]),
                                    start=(kj == 0), stop=(kj == 3))
                        fb = fbnc.tile([128, L], f32, tag="fb")
                        nc.vector.tensor_copy(fb, pf)
                        nc.sync.dma_start(out=out_d[m16 * 128:(m16 + 1) * 128, :], in_=fb)
    return nc


_NC_CACHE = None


def kernel(**inputs):
    global _NC_CACHE
    from concourse.bass_utils import run_bass_kernel_spmd

    in_maps = _host_prep(
        np.asarray(inputs["x"], np.float32), np.asarray(inputs["condition"], np.float32),
        np.asarray(inputs["rope"], np.float32), np.asarray(inputs["w_ada"], np.float32),
        np.asarray(inputs["w_qkv"], np.float32), np.asarray(inputs["w_out"], np.float32),
        np.asarray(inputs["qk_w"], np.float32))
    if _NC_CACHE is None:
        _NC_CACHE = _build_nc()
        if not _NC_CACHE.is_finalized():
            _NC_CACHE.finalize()
    res = run_bass_kernel_spmd(_NC_CACHE, in_maps, list(range(NCORES)))
    out = np.zeros((B, L, D), np.float32)
    for b in range(B):
        acc = np.zeros((D, L), np.float32)
        for g in range(4):
            acc += res.results[b * 4 + g]["out"]
        out[b] = acc.T
    return out


# revision 18
# speedup vs baseline: 7935.7412x; 7935.7412x over previous
"""Trainium2 Bass kernel for fused AdaRMSNorm + QK-RMSNorm/RoPE attention.

Sharding: 8 cores = 2 batch groups x 4 head-groups (8 heads each).
Device math (per core, batch b, head-group g):
  xs = x[b].T * rsqrt(mean_d(x^2)+eps)            (adaRMS: scale folded into W, shift into bias c)
  qT/kT = (W*.T)^T @ xs + c   (hd-major)           v = xs.T @ Wv* + cv  (L-major)
  per-head: qn = q * rsqrt(mean_hd(q^2)+eps); rope via partition pair-shuffle,
            qk_w/sign/(1/sqrt hd) folded into host rope tables
  S^T = kr^T qr ; E^T = exp(S^T) (no max-sub: scores are O(5))
  o+ = [v;1]^T E^T  -> oT = o/denom ; partial outT = W_o_loc @ oT
Host: final 4-way partial sum per batch + transpose.
"""

import numpy as np

B, L, D, HD, DC = 2, 2048, 2048, 64, 2048
NH = D // HD
EPS = float(np.finfo(np.float32).eps)
NCORES = 8
JL = 512          # local j per device (8 heads x 64)
NHL = 8           # local heads


# ---------------------------------------------------------------- host prep
def _host_prep(x, condition, rope, w_ada, w_qkv, w_out, qk_w):
    hd_idx = np.arange(HD)
    sign = np.where(hd_idx % 2 == 0, -1.0, 1.0).astype(np.float32)
    partner = hd_idx ^ 1
    rope0, rope1 = rope[0].T, rope[1].T                      # [64, L]
    ropeA = (rope0 * qk_w[:, None]).astype(np.float32)
    ropeB = (rope1 * (sign * qk_w[partner])[:, None]).astype(np.float32)
    scl = np.float32(1.0 / np.sqrt(HD))
    # 2-head replicated [128, L] tiles
    ropeA2 = np.tile(ropeA, (2, 1))
    ropeB2 = np.tile(ropeB, (2, 1))
    ra_q = np.ascontiguousarray(ropeA2 * scl)
    rb_q = np.ascontiguousarray(ropeB2 * scl)
    ra_k = np.ascontiguousarray(ropeA2)
    rb_k = np.ascontiguousarray(ropeB2)

    smat = np.zeros((JL, NHL), np.float32)                   # j -> head one-hot
    for j in range(JL):
        smat[j, j // HD] = 1.0
    sbmat = np.ascontiguousarray(smat.T)                     # [8, 512]

    in_maps = []
    for dev in range(NCORES):
        b, g = dev // 4, dev % 4
        ss = (w_ada @ condition[b]).astype(np.float32)
        shift, s1 = ss[:D], (1.0 + ss[D:]).astype(np.float32)
        Wq = w_qkv[g * JL:(g + 1) * JL]
        Wk = w_qkv[D + g * JL:D + (g + 1) * JL]
        Wv = w_qkv[2 * D + g * JL:2 * D + (g + 1) * JL]
        m = {
            "xT": np.ascontiguousarray(x[b].T),
            "wq": np.ascontiguousarray((Wq * s1[None, :]).T),
            "wk": np.ascontiguousarray((Wk * s1[None, :]).T),
            "wv": np.ascontiguousarray((Wv * s1[None, :]).T),
            "cq": (Wq @ shift).astype(np.float32).reshape(1, JL),
            "ck": (Wk @ shift).astype(np.float32).reshape(1, JL),
            "cv": (Wv @ shift).astype(np.float32).reshape(1, JL),
            "ra_q": ra_q, "rb_q": rb_q, "ra_k": ra_k, "rb_k": rb_k,
            "smat": smat, "sbmat": sbmat,
            "woT": np.ascontiguousarray(w_out[:, g * JL:(g + 1) * JL].T),  # [512 j, 2048 dcol]
        }
        in_maps.append(m)
    return in_maps


# ---------------------------------------------------------------- bass build
def _build_nc():
    import concourse.bass as bass
    import concourse.mybir as mybir
    import concourse.tile as tile
    from concourse import bacc

    f32 = mybir.dt.float32
    f32r = mybir.dt.float32r
    PS = 512
    AF = mybir.ActivationFunctionType

    nc = bacc.Bacc("TRN2", target_bir_lowering=False, debug=False, num_devices=8)
    xT_d = nc.dram_tensor("xT", [D, L], f32, kind="ExternalInput")
    w_d = {nm: nc.dram_tensor(nm, [D, JL], f32, kind="ExternalInput")
           for nm in ("wq", "wk", "wv")}
    c_d = {nm: nc.dram_tensor(f"c{nm}", [1, JL], f32, kind="ExternalInput")
           for nm in ("q", "k", "v")}
    rope_d = {nm: nc.dram_tensor(nm, [128, L], f32, kind="ExternalInput")
              for nm in ("ra_q", "rb_q", "ra_k", "rb_k")}
    smat_d = nc.dram_tensor("smat", [JL, NHL], f32, kind="ExternalInput")
    sbmat_d = nc.dram_tensor("sbmat", [NHL, JL], f32, kind="ExternalInput")
    woT_d = nc.dram_tensor("woT", [JL, D], f32, kind="ExternalInput")
    out_d = nc.dram_tensor("out", [D, L], f32, kind="ExternalOutput")

    import os
    USE_F32R = os.environ.get("USE_F32R", "1") == "1"

    def r(ap):
        return ap.bitcast(f32r) if USE_F32R else ap

    import contextlib
    lp = nc.allow_low_precision(reason="fp32r matmul operands") if USE_F32R \
        else contextlib.nullcontext()
    with tile.TileContext(nc) as tc, lp:
        with (
            tc.tile_pool(name="consts", bufs=1) as consts,
            tc.tile_pool(name="dram", bufs=1, space="DRAM") as dpool,
        ):
            ones_raw = consts.tile([128, PS], f32)
            nc.vector.memset(ones_raw, 1.0)
            ones = consts.tile([128, PS], f32)
            nc.vector.tensor_copy(r(ones), ones_raw)
            eps1 = consts.tile([1, 1], f32)
            nc.vector.memset(eps1, EPS)
            eps8 = consts.tile([8, 1], f32)
            nc.vector.memset(eps8, EPS)
            s_t = consts.tile([128, 4, NHL], f32)
            for i in range(4):
                nc.sync.dma_start(out=r(s_t[:, i, :]), in_=r(smat_d[i * 128:(i + 1) * 128, :]))
            sb_t = consts.tile([NHL, JL], f32)
            nc.sync.dma_start(out=r(sb_t), in_=r(sbmat_d[:, :]))
            c_t = {}
            for nm in ("q", "k", "v"):
                c_t[nm] = consts.tile([1, JL], f32, tag=f"c{nm}", name=f"c{nm}")
                nc.sync.dma_start(out=r(c_t[nm]), in_=r(c_d[nm][:, :]))
            rr_row = consts.tile([1, L], f32)

            qT_sp = dpool.tile([JL, L], f32, tag="qTsp")
            kT_sp = dpool.tile([JL, L], f32, tag="kTsp")
            v_sp = dpool.tile([L, NHL, HD + 1], f32, tag="vsp")

            # ================= Phase A/B/C: xs resident ====================
            with tc.tile_pool(name="xs", bufs=1) as xsp:
                xs = []
                for i in range(16):
                    t = xsp.tile([128, L], f32, tag=f"xs{i}", name=f"xs{i}")
                    nc.sync.dma_start(out=r(t), in_=r(xT_d[i * 128:(i + 1) * 128, :]))
                    xs.append(t)
                # ---- A: sumsq over d via ones-matmul; B: xs *= rr ----
                with (
                    tc.tile_pool(name="tmpA", bufs=2) as tmpA,
                    tc.tile_pool(name="psA", bufs=1, space="PSUM") as psA,
                ):
                    ssq = psA.tile([1, L], f32, tag="ssq")
                    touch = tmpA.tile([1, 16], f32, tag="touch", name="touch", bufs=2)
                    for i in range(16):
                        nc.vector.tensor_copy(touch[0:1, i:i + 1], xs[i][0:1, 0:1])
                        for hh in range(2):
                            sq = tmpA.tile([128, 1024], f32, tag="sq")
                            sl = slice(hh * 1024, (hh + 1) * 1024)
                            nc.vector.tensor_mul(r(sq), xs[i][:, sl], xs[i][:, sl])
                            for n2 in range(2):
                                nc.tensor.matmul(
                                    ssq[:, hh * 1024 + n2 * PS:hh * 1024 + (n2 + 1) * PS],
                                    lhsT=r(ones[:, 0:1]), rhs=r(sq[:, n2 * PS:(n2 + 1) * PS]),
                                    start=(i == 0), stop=(i == 15))
                    nc.scalar.activation(r(rr_row), ssq, AF.Sqrt, bias=eps1, scale=1.0 / D)
                    nc.vector.reciprocal(r(rr_row), rr_row)
                    rrb_ps = psA.tile([128, L], f32, tag="rrb")
                    for n in range(4):
                        nc.tensor.matmul(rrb_ps[:, n * PS:(n + 1) * PS],
                                         lhsT=r(ones[0:1, 0:128]),
                                         rhs=r(rr_row[:, n * PS:(n + 1) * PS]),
                                         start=True, stop=True)
                    for i in range(16):
                        nc.vector.tensor_mul(r(xs[i]), xs[i], rrb_ps)

                # ---- C1: q/k projections (hd-major), spill to DRAM ----
                with (
                    tc.tile_pool(name="wqk", bufs=1) as wqk,
                    tc.tile_pool(name="bnc", bufs=2) as bnc,
                    tc.tile_pool(name="psC", bufs=1, space="PSUM") as psC,
                ):
                    for nm, spill in (("q", qT_sp), ("k", kT_sp)):
                        for lh in range(2):
                            ps = [psC.tile([128, 1024], f32, tag=f"pj{m}", name=f"pj{m}")
                                  for m in range(4)]
                            for k16 in range(16):
                                wt = wqk.tile([128, JL], f32, tag=f"w{k16 % 8}",
                                              name=f"w{k16 % 8}")
                                nc.sync.dma_start(
                                    out=r(wt), in_=r(w_d["w" + nm][k16 * 128:(k16 + 1) * 128, :]))
                                for m in range(4):
                                    for n2 in range(2):
                                        nc.tensor.matmul(
                                            ps[m][:, n2 * PS:(n2 + 1) * PS],
                                            lhsT=r(wt[:, m * 128:(m + 1) * 128]),
                                            rhs=r(xs[k16][:, lh * 1024 + n2 * PS:
                                                          lh * 1024 + (n2 + 1) * PS]),
                                            start=(k16 == 0), stop=False)
                            for m in range(4):
                                for n2 in range(2):
                                    nc.tensor.matmul(
                                        ps[m][:, n2 * PS:(n2 + 1) * PS],
                                        lhsT=r(c_t[nm][:, m * 128:(m + 1) * 128]),
                                        rhs=r(ones[0:1, 0:PS]), start=False, stop=True)
                                o = bnc.tile([128, 1024], f32, tag="bounce")
                                nc.vector.tensor_copy(o, ps[m])
                                nc.sync.dma_start(
                                    out=spill[m * 128:(m + 1) * 128, lh * 1024:(lh + 1) * 1024],
                                    in_=o)

                # ---- C2: v projection (L-major), spill with ones column ----
                with (
                    tc.tile_pool(name="wv16", bufs=1) as wv16,
                    tc.tile_pool(name="bncv", bufs=2) as bncv,
                    tc.tile_pool(name="psV", bufs=2, space="PSUM") as psV,
                ):
                    wt = []
                    for k16 in range(16):
                        t = wv16.tile([128, JL], f32, tag=f"wv{k16}", name=f"wv{k16}")
                        nc.sync.dma_start(out=r(t), in_=r(w_d["wv"][k16 * 128:(k16 + 1) * 128, :]))
                        wt.append(t)
                    for mL in range(16):
                        pv = psV.tile([128, JL], f32, tag="pv")
                        for k16 in range(16):
                            nc.tensor.matmul(pv, lhsT=r(xs[k16][:, mL * 128:(mL + 1) * 128]),
                                             rhs=r(wt[k16]), start=(k16 == 0), stop=False)
                        nc.tensor.matmul(pv, lhsT=r(ones[0:1, 0:128]), rhs=r(c_t["v"]),
                                         start=False, stop=True)
                        vb = bncv.tile([128, NHL, HD + 1], f32, tag="vb")
                        nc.vector.tensor_copy(vb[:, :, 0:HD],
                                              pv.rearrange("p (h d) -> p h d", h=NHL))
                        nc.vector.memset(vb[:, :, HD:HD + 1], 1.0)
                        nc.sync.dma_start(out=v_sp[mL * 128:(mL + 1) * 128], in_=vb)

            # ================= Phase D/E/F =================================
            with tc.tile_pool(name="oT", bufs=1) as otp:
                oT = [otp.tile([128, L], f32, tag=f"oT{m}", name=f"oT{m}") for m in range(4)]
                with tc.tile_pool(name="qr", bufs=1) as qrp:
                    qr_t, kr_t = [], []
                    # ---- D: per-head RMS + RoPE ----
                    with (
                        tc.tile_pool(name="ropes", bufs=1) as rp,
                        tc.tile_pool(name="dtmp", bufs=2) as dtmp,
                        tc.tile_pool(name="psD", bufs=1, space="PSUM") as psD,
                    ):
                        rope_t = {}
                        for nm in ("ra_q", "rb_q", "ra_k", "rb_k"):
                            rope_t[nm] = rp.tile([128, L], f32, tag=nm, name=nm)
                            nc.sync.dma_start(out=rope_t[nm], in_=rope_d[nm][:, :])
                        shuf = [i ^ 1 for i in range(32)]
                        for is_q, spill, dst in ((1, qT_sp, qr_t), (0, kT_sp, kr_t)):
                            ra = rope_t["ra_q" if is_q else "ra_k"]
                            rb = rope_t["rb_q" if is_q else "rb_k"]
                            for m in range(4):
                                qk = dtmp.tile([128, L], f32, tag="tA", name="tA", bufs=2)
                                nc.sync.dma_start(out=qk, in_=spill[m * 128:(m + 1) * 128, :])
                                sq = dtmp.tile([128, L], f32, tag="tB", name="tB", bufs=2)
                                nc.scalar.square(r(sq), qk)
                                ph = psD.tile([NHL, L], f32, tag="ph")
                                for n in range(4):
                                    nc.tensor.matmul(ph[:, n * PS:(n + 1) * PS],
                                                     lhsT=r(s_t[:, m, :]),
                                                     rhs=r(sq[:, n * PS:(n + 1) * PS]),
                                                     start=True, stop=True)
                                rrh = dtmp.tile([NHL, L], f32, tag="tC", name="tC", bufs=1)
                                nc.scalar.activation(r(rrh), ph, AF.Sqrt, bias=eps8, scale=1.0 / HD)
                                nc.vector.reciprocal(r(rrh), rrh)
                                pb = psD.tile([128, L], f32, tag="pb")
                                for n in range(4):
                                    nc.tensor.matmul(pb[:, n * PS:(n + 1) * PS],
                                                     lhsT=r(sb_t[:, m * 128:(m + 1) * 128]),
                                                     rhs=r(rrh[:, n * PS:(n + 1) * PS]),
                                                     start=True, stop=True)
                                qn = dtmp.tile([128, L], f32, tag="tD", name="tD", bufs=1)
                                nc.vector.tensor_mul(qn, qk, pb)
                                qs = dtmp.tile([128, L], f32, tag="tB", name="qs", bufs=2)
                                nc.vector.stream_shuffle(qs, qn, shuf)
                                t1 = dtmp.tile([128, L], f32, tag="tC", name="t1", bufs=1)
                                nc.vector.tensor_mul(t1, qn, ra)
                                t2 = dtmp.tile([128, L], f32, tag="tA", name="t2", bufs=2)
                                nc.gpsimd.tensor_mul(t2, qs, rb)
                                res = qrp.tile([128, L], f32,
                                               tag=f"{'qr' if is_q else 'kr'}{m}",
                                               name=f"{'qr' if is_q else 'kr'}{m}")
                                nc.vector.tensor_add(r(res), t1, t2)
                                dst.append(res)

                    # ---- E: attention ----
                    with (
                        tc.tile_pool(name="vload", bufs=1) as vlp,
                        tc.tile_pool(name="et", bufs=3) as etp,
                        tc.tile_pool(name="etmp", bufs=2) as etmp,
                        tc.tile_pool(name="psS", bufs=2, space="PSUM") as psS,
                        tc.tile_pool(name="psO", bufs=1, space="PSUM") as psO,
                    ):
                        for h in range(NHL):
                            qr_h = qr_t[h // 2][(h % 2) * 64:(h % 2) * 64 + 64, :]
                            kr_h = kr_t[h // 2][(h % 2) * 64:(h % 2) * 64 + 64, :]
                            vt = []
                            for l2c in range(16):
                                t = vlp.tile([128, HD + 1], f32, tag=f"v{l2c}",
                                             name=f"v{l2c}")
                                nc.sync.dma_start(out=r(t),
                                                  in_=r(v_sp[l2c * 128:(l2c + 1) * 128, h, :]))
                                vt.append(t)
                            for lhf in range(2):
                                po = psO.tile([HD + 1, 1024], f32, tag="po")
                                for l2c in range(16):
                                    pS = psS.tile([128, 1024], f32, tag="pS", name="pS")
                                    for n2 in range(2):
                                        nc.tensor.matmul(
                                            pS[:, n2 * PS:(n2 + 1) * PS],
                                            lhsT=r(kr_h[:, l2c * 128:(l2c + 1) * 128]),
                                            rhs=r(qr_h[:, lhf * 1024 + n2 * PS:
                                                       lhf * 1024 + (n2 + 1) * PS]),
                                            start=True, stop=True)
                                    et = etp.tile([128, 1024], f32, tag="et", name="et")
                                    nc.scalar.activation(r(et), pS, AF.Exp)
                                    for n2 in range(2):
                                        nc.tensor.matmul(
                                            po[:, n2 * PS:(n2 + 1) * PS],
                                            lhsT=r(vt[l2c]),
                                            rhs=r(et[:, n2 * PS:(n2 + 1) * PS]),
                                            start=(l2c == 0), stop=(l2c == 15))
                                rd = etmp.tile([1, 1024], f32, tag="rd", name="rd")
                                nc.vector.reciprocal(r(rd), po[HD:HD + 1, :])
                                prd = psS.tile([128, 1024], f32, tag="pS", name="prd")
                                for n2 in range(2):
                                    nc.tensor.matmul(prd[0:HD, n2 * PS:(n2 + 1) * PS],
                                                     lhsT=r(ones[0:1, 0:HD]),
                                                     rhs=r(rd[:, n2 * PS:(n2 + 1) * PS]),
                                                     start=True, stop=True)
                                ob = etmp.tile([HD, 1024], f32, tag="ob", name="ob")
                                nc.scalar.copy(ob, po[0:HD, :])
                                nc.vector.tensor_mul(
                                    r(oT[h // 2][(h % 2) * 64:(h % 2) * 64 + 64,
                                                 lhf * 1024:(lhf + 1) * 1024]),
                                    ob, prd[0:HD, :])

                # ---- F: partial output projection ----
                with (
                    tc.tile_pool(name="wo", bufs=1) as wop,
                    tc.tile_pool(name="fbnc", bufs=2) as fbnc,
                    tc.tile_pool(name="psF", bufs=2, space="PSUM") as psF,
                ):
                    wo = []
                    for kj in range(4):
                        t = wop.tile([128, D], f32, tag=f"wo{kj}", name=f"wo{kj}")
                        nc.sync.dma_start(out=r(t), in_=r(woT_d[kj * 128:(kj + 1) * 128, :]))
                        wo.append(t)
                    for m16 in range(16):
                        pf = psF.tile([128, L], f32, tag="pf")
                        for kj in range(4):
                            for n in range(4):
                                nc.tensor.matmul(
                                    pf[:, n * PS:(n + 1) * PS],
                                    lhsT=r(wo[kj][:, m16 * 128:(m16 + 1) * 128]),
                                    rhs=r(oT[kj][:, n * PS:(n + 1) * PS]),
                                    start=(kj == 0), stop=(kj == 3))
                        fb = fbnc.tile([128, L], f32, tag="fb")
                        nc.vector.tensor_copy(fb, pf)
                        nc.sync.dma_start(out=out_d[m16 * 128:(m16 + 1) * 128, :], in_=fb)
    return nc


_NC_CACHE = None


def kernel(**inputs):
    global _NC_CACHE
    from concourse.bass_utils import run_bass_kernel_spmd

    in_maps = _host_prep(
        np.asarray(inputs["x"], np.float32), np.asarray(inputs["condition"], np.float32),
        np.asarray(inputs["rope"], np.float32), np.asarray(inputs["w_ada"], np.float32),
        np.asarray(inputs["w_qkv"], np.float32), np.asarray(inputs["w_out"], np.float32),
        np.asarray(inputs["qk_w"], np.float32))
    if _NC_CACHE is None:
        _NC_CACHE = _build_nc()
        if not _NC_CACHE.is_finalized():
            _NC_CACHE.finalize()
    res = run_bass_kernel_spmd(_NC_CACHE, in_maps, list(range(NCORES)))
    out = np.zeros((B, L, D), np.float32)
    for b in range(B):
        acc = np.zeros((D, L), np.float32)
        for g in range(4):
            acc += res.results[b * 4 + g]["out"]
        out[b] = acc.T
    return out
